# revision 13
# baseline (speedup 1.0000x reference)
"""BiDAF attention kernel for trn2 (8 NeuronCores, pure data parallel).

v4: DMA/tensor co-optimized.
- S (and its Aq = w^T q^T prefactor) stay exact fp32: the softmax over LQ
  is effectively a hard max on ~N(0,256^2) logits, so bf16/tf32 rounding
  flips winners (measured rel err 0.49 / 0.06 -- both over the 2e-2 gate).
- Everything downstream of the exp is bf16 on the tensor engine (1 cy/row):
  the a-transpose, the U matmul, and the Ht broadcast.
- p is loaded ONCE per batch (natural layout, directly into the output
  assembly tile Gt); p^T is derived on-chip via tensor-engine transposes.
- h = P @ softmax(max_q S) is computed in column form: 8 accumulating
  N=1 fp32 matmuls + 2 small transposes, instead of a 257-row fp32 matmul.
- The a-normalization is folded into the P*U multiply via
  scalar_tensor_tensor((U_un * 1/rs) * P) on DVE.
- The Ht output block is never materialized per-row: a stride-0
  (broadcast_to) DMA source writes the same 100x256 tile to all 4 row
  chunks of g[:, :, 256:512].
- Loads issue on the Activation (scalar) DMA queue, stores on the SP (sync)
  queue, so loads never serialize behind compute-blocked stores.
"""

from contextlib import ExitStack

import numpy as np
import ml_dtypes

import concourse.bass as bass
import concourse.mybir as mybir
import concourse.tile as tile
from concourse.bass_utils import run_bass_kernel_spmd
from concourse.masks import make_identity

F32 = mybir.dt.float32
F32R = mybir.dt.float32r
BF16 = mybir.dt.bfloat16
AX = mybir.AxisListType
ALU = mybir.AluOpType
ACTF = mybir.ActivationFunctionType

B, LP, LQ, H = 128, 400, 100, 256
NCORES = 8
BP = B // NCORES       # batches per core
R = 100                # rows per LP chunk
NCH = LP // R          # 4 chunks


def build_nc():
    nc = bass.Bass("TRN2", target_bir_lowering=False, debug=False)

    pn = nc.dram_tensor("pn", [BP, LP, H], F32, kind="ExternalInput")
    qtg = nc.dram_tensor("qtg", [2, 128, BP, LQ], F32, kind="ExternalInput")
    qnb = nc.dram_tensor("qnb", [BP, LQ, H], BF16, kind="ExternalInput")
    w = nc.dram_tensor("w", [H, H], F32, kind="ExternalInput")
    g = nc.dram_tensor("g", [BP, LP, 4 * H], F32, kind="ExternalOutput")

    with tile.TileContext(nc) as tc, ExitStack() as ctx:
        cpool = ctx.enter_context(tc.tile_pool(name="consts", bufs=1))
        wp = ctx.enter_context(tc.tile_pool(name="work", bufs=2))
        pp = ctx.enter_context(tc.tile_pool(name="ps", bufs=2, space="PSUM"))

        # ---- constants ----
        identf = cpool.tile([128, 128], F32)
        make_identity(nc, identf[:])
        identb = cpool.tile([128, 128], BF16)
        make_identity(nc, identb[:])
        onesM = cpool.tile([1, 128], F32)
        nc.vector.memset(onesM[:], 1.0)
        onesMb = cpool.tile([1, 128], BF16)
        nc.vector.memset(onesMb[:], 1.0)
        ones_c = cpool.tile([128, 1], F32)
        nc.vector.memset(ones_c[:], 1.0)
        Wt = cpool.tile([128, 2, H], F32)
        nc.sync.dma_start(Wt[:], w.rearrange("(k p) h -> p k h", p=128))

        # ---- prepass: load q in both layouts, compute Aq = w^T q^T ----
        QtA = cpool.tile([128, 2, BP * LQ], F32)
        nc.sync.dma_start(QtA[:], qtg.rearrange("k p b l -> p k (b l)"))
        QnB = cpool.tile([128, BP, H], BF16)
        nc.sync.dma_start(QnB[0:LQ, :, :], qnb.rearrange("b l h -> l b h"))

        AqA = cpool.tile([128, 2, BP * LQ], F32)
        for gi in range(BP // 4):
            for ms in range(2):
                psAq = pp.tile([128, 400], F32, tag="pt")
                for kc in range(2):
                    nc.tensor.matmul(
                        psAq[:],
                        Wt[:, kc, ms * 128:(ms + 1) * 128],
                        QtA[:, kc, gi * 400:(gi + 1) * 400],
                        start=(kc == 0), stop=(kc == 1),
                    )
                dst = AqA[:, ms, gi * 400:(gi + 1) * 400]
                if ms == 0:
                    nc.scalar.copy(dst, psAq[:])
                else:
                    nc.vector.tensor_copy(dst, psAq[:])

        # ---- main loop over batches ----
        # Loads are emitted PF batches ahead on the same (sync) queue so a
        # load never sits behind a compute-blocked store at the queue head.
        PF = 4
        pend = {}

        def emit_load(bb):
            Gtn = wp.tile([128, NCH, 3 * H], F32, tag="Gt", bufs=PF + 1,
                          name=f"Gt{bb}")
            nc.sync.dma_start(
                Gtn[0:R, :, 0:H],
                pn[bb].rearrange("(i r) h -> r i h", r=R))
            pend[bb] = Gtn

        for bb in range(PF):
            emit_load(bb)

        for b in range(BP):
            bq = b * LQ
            if b + PF < BP:
                emit_load(b + PF)
            Gt = pend.pop(b)

            # ---- p^T via tensor transposes ----
            psPt = [pp.tile([128, 400], F32, tag="pt", name=f"psPt{k}")
                    for k in range(2)]
            for kc in range(2):
                for i in range(NCH):
                    nc.tensor.transpose(
                        psPt[kc][0:128, i * R:(i + 1) * R],
                        Gt[0:R, i, kc * 128:(kc + 1) * 128],
                        identf[0:R, 0:R])
            PtS = wp.tile([128, 2, 400], F32, tag="PtS")
            nc.scalar.copy(PtS[:, 0, :], psPt[0][:])
            nc.vector.tensor_copy(PtS[:, 1, :], psPt[1][:])

            # ---- S^T = p @ Aq  (LP chunks on partitions, LQ free) ----
            psS = pp.tile([128, NCH, LQ], F32, tag="st")
            for i in range(NCH):
                for kc in range(2):
                    nc.tensor.matmul(
                        psS[0:R, i, :],
                        PtS[:, kc, i * R:(i + 1) * R],
                        AqA[:, kc, bq:bq + LQ],
                        start=(kc == 0), stop=(kc == 1),
                    )

            # ---- C2Q softmax over LQ ----
            NM = wp.tile([128, NCH], F32, tag="NM")
            for i in range(NCH):
                nc.vector.tensor_reduce(
                    NM[0:R, i:i + 1], psS[0:R, i, :],
                    axis=AX.X, op=ALU.max, negate=True)
            E = wp.tile([128, NCH, LQ], BF16, tag="E")
            RS = wp.tile([128, NCH], F32, tag="RS")
            for i in range(NCH):
                nc.scalar.activation(
                    E[0:R, i, :], psS[0:R, i, :], ACTF.Exp,
                    bias=NM[0:R, i:i + 1], accum_out=RS[0:R, i:i + 1])
            RCP = wp.tile([128, NCH], F32, tag="RCP")
            nc.vector.reciprocal(RCP[0:R, :], RS[0:R, :])

            # ---- transpose e^T -> e (bf16) ----
            psAm = pp.tile([128, NCH, LQ], BF16, tag="st")
            for i in range(NCH):
                nc.tensor.transpose(
                    psAm[0:LQ, i, 0:R], E[0:R, i, :], identb[0:R, 0:R])
            Am = wp.tile([128, NCH, R], BF16, tag="Am")
            nc.vector.tensor_copy(Am[0:LQ, :, :], psAm[0:LQ, :, :])

            # ---- U^T chunks + fused normalize * P ----
            for i in range(NCH):
                psU = pp.tile([128, H], F32, tag="u")
                nc.tensor.matmul(
                    psU[0:R, :], Am[0:LQ, i, :], QnB[0:LQ, b, :],
                    start=True, stop=True)
                nc.vector.scalar_tensor_tensor(
                    Gt[0:R, i, H:2 * H], psU[0:R, :], RCP[0:R, i:i + 1],
                    Gt[0:R, i, 0:H], ALU.mult, ALU.mult)

            # ---- Q2C: global max, weights, attended row h ----
            psTn = pp.tile([128, 512], F32, tag="tn")
            nm1 = wp.tile([128, 1], F32, tag="nm1")
            nc.vector.tensor_reduce(nm1[0:R, :], NM[0:R, :], axis=AX.X,
                                    op=ALU.min)
            nc.tensor.transpose(psTn[0:1, 0:R], nm1[0:R, 0:1],
                                identf[0:R, 0:R])
            ngb = wp.tile([1, 1], F32, tag="ngb")
            nc.vector.tensor_reduce(ngb[0:1, :], psTn[0:1, 0:R], axis=AX.X,
                                    op=ALU.min)
            nc.tensor.matmul(psTn[0:R, 128:129], onesM[0:1, 0:R],
                             ngb[0:1, 0:1], start=True, stop=True)
            nb = wp.tile([128, 1], F32, tag="nb")
            nc.scalar.copy(nb[0:R, :], psTn[0:R, 128:129])
            EQ = wp.tile([128, NCH], F32, tag="EQ")
            nc.scalar.activation(EQ[0:R, :], NM[0:R, :], ACTF.Exp,
                                 bias=nb[0:R, 0:1], scale=-1.0)
            eqs = wp.tile([128, 1], F32, tag="eqs")
            nc.vector.tensor_reduce(eqs[0:R, :], EQ[0:R, :], axis=AX.X,
                                    op=ALU.add)
            nc.tensor.matmul(psTn[0:1, 200:201], eqs[0:R, 0:1],
                             ones_c[0:R, 0:1], start=True, stop=True)
            # h in column form: psTn[:, 132:134] accumulates h over chunks
            for c in range(2):
                for i in range(NCH):
                    nc.tensor.matmul(
                        psTn[0:128, 132 + c:133 + c],
                        Gt[0:R, i, c * 128:(c + 1) * 128],
                        EQ[0:R, i:i + 1],
                        start=(i == 0), stop=(i == NCH - 1))
            h2 = wp.tile([128, 2], F32, tag="h2")
            nc.scalar.copy(h2[0:128, :], psTn[0:128, 132:134])
            for c in range(2):
                nc.tensor.transpose(psTn[0:1, 256 + c * 128:384 + c * 128],
                                    h2[0:128, c:c + 1], identf[:])
            rq = wp.tile([1, 1], F32, tag="rq")
            nc.vector.reciprocal(rq[0:1, :], psTn[0:1, 200:201])
            hrowb = wp.tile([1, H], BF16, tag="hrowb")
            nc.scalar.mul(hrowb[0:1, :], psTn[0:1, 256:512], rq[0:1, 0:1])
            psHt = pp.tile([128, H], F32, tag="u")
            nc.tensor.matmul(psHt[0:R, :], onesMb[0:1, 0:R],
                             hrowb[0:1, :], start=True, stop=True)
            HtS = wp.tile([128, 1, H], F32, tag="HtS")
            nc.scalar.copy(HtS[0:R, 0, :], psHt[0:R, :])

            # ---- P * Ht ----
            for i in range(NCH):
                eng = nc.gpsimd if i < 3 else nc.vector
                eng.tensor_tensor(
                    Gt[0:R, i, 2 * H:3 * H], Gt[0:R, i, 0:H],
                    HtS[0:R, 0, :], op=ALU.mult)

            # ---- stores ----
            gv = g[b].rearrange("(i r) h -> r i h", r=R)
            nc.sync.dma_start(gv[:, :, 0:H], Gt[0:R, :, 0:H])
            nc.sync.dma_start(
                gv[:, :, H:2 * H],
                HtS[0:R, 0:1, 0:H].broadcast_to([R, NCH, H]))
            nc.sync.dma_start(gv[:, :, 2 * H:4 * H], Gt[0:R, :, H:3 * H])

    return nc


def legalize_waits(nc):
    """Split multi-wait instructions into single-wait NoOps + instruction.

    The TPB ISA has exactly one (wait, update) EVENTS slot per 64B
    instruction; this walrus build refuses instructions with more than one
    sync wait ("Too many sync wait commands").  Tile's scheduler emits
    vector-clock waits freely, so legalize here: excess waits move onto
    engine-queue NoOps placed immediately before the instruction.
    """
    counter = 0
    for f in nc.m.functions:
        for blk in f.blocks:
            new = []
            for inst in blk.instructions:
                si = getattr(inst, "sync_info", None)
                if si is not None and len(si.on_wait) > 1:
                    waits = list(si.on_wait)
                    assert len(si.on_update) <= 1, inst
                    for wt in waits[:-1]:
                        counter += 1
                        new.append(mybir.InstNoOp(
                            name=f"I-waitnop-{counter}",
                            engine=inst.engine,
                            sync_info=mybir.SyncInfo(on_wait=[wt],
                                                     on_update=[]),
                        ))
                    inst.sync_info = mybir.SyncInfo(
                        on_wait=[waits[-1]], on_update=list(si.on_update))
                new.append(inst)
            blk.instructions = new
    return nc


def _make_in_maps(p, q, w):
    p = np.ascontiguousarray(p, dtype=np.float32)
    q = np.ascontiguousarray(q, dtype=np.float32)
    w = np.ascontiguousarray(w, dtype=np.float32)
    in_maps = []
    for c in range(NCORES):
        sl = slice(c * BP, (c + 1) * BP)
        qc = q[sl]
        in_maps.append({
            "pn": p[sl],
            "qtg": np.ascontiguousarray(
                qc.transpose(2, 0, 1).reshape(2, 128, BP, LQ)),
            "qnb": np.ascontiguousarray(qc.astype(ml_dtypes.bfloat16)),
            "w": w,
        })
    return in_maps


def run(p, q, w, trace=False):
    nc = legalize_waits(build_nc())
    res = run_bass_kernel_spmd(
        nc, _make_in_maps(p, q, w), list(range(NCORES)), trace=trace)
    out = np.concatenate([res.results[c]["g"] for c in range(NCORES)], axis=0)
    return out, res


def kernel(p, q, w):
    out, _ = run(p, q, w, trace=False)
    return out


# revision 16
# speedup vs baseline: 1.5275x; 1.5275x over previous
"""BiDAF attention kernel for trn2 (8 NeuronCores, pure data parallel).

v5: software-pipelined emission, minimal tensor instruction count.
- S (and Aq = w^T q^T) stay exact fp32: the C2Q softmax is a near-hard max
  on ~N(0,256^2) logits; bf16/tf32 rounding flips winners (0.49 / 0.06 rel
  err, both over the 2e-2 gate).  fp32 matmuls are walrus-split into 2
  instructions; everything else runs bf16 (1 cy/row, single instruction).
- p loads ONCE per batch straight into the output tile Gt; p^T derived
  on-chip (8 tensor transposes/batch).
- h = P @ softmax(max_q S) in bf16 row form against a bf16 P copy with a
  fused ones column (gives the normalizer for free).
- Stabilizer paths (global max g, Ht broadcast) run bf16: g cancels in the
  softmax ratio, Ht tolerates 4e-3.
- Emission is a 5-stage software pipeline (load+5 / transpose+3 / S+2 /
  U,h+0 / softmax+1 / broadcast,store-1) so no tensor instruction waits on
  a same-iteration cross-engine result: the PE queue never stalls, which
  also keeps the HAM clock gate open.
- The Ht output block is written by a stride-0 (broadcast_to) DMA source,
  never materialized per-row in SBUF.
- PSUM: 8 banks exactly (pt x2, st x2, am x1, u x2, glue x1 persistent
  with even/odd-batch column halves).
"""

from contextlib import ExitStack

import numpy as np
import ml_dtypes

import concourse.bass as bass
import concourse.mybir as mybir
import concourse.tile as tile
from concourse.bass_utils import run_bass_kernel_spmd
from concourse.masks import make_identity

F32 = mybir.dt.float32
BF16 = mybir.dt.bfloat16
AX = mybir.AxisListType
ALU = mybir.AluOpType
ACTF = mybir.ActivationFunctionType

B, LP, LQ, H = 128, 400, 100, 256
NCORES = 8
BP = B // NCORES       # batches per core
R = 100                # rows per LP chunk
NCH = LP // R          # 4 chunks


def build_nc():
    nc = bass.Bass("TRN2", target_bir_lowering=False, debug=False)

    pn = nc.dram_tensor("pn", [BP, LP, H], F32, kind="ExternalInput")
    qtg = nc.dram_tensor("qtg", [2, 128, BP, LQ], F32, kind="ExternalInput")
    qnb = nc.dram_tensor("qnb", [BP, LQ, H], BF16, kind="ExternalInput")
    w = nc.dram_tensor("w", [H, H], F32, kind="ExternalInput")
    g = nc.dram_tensor("g", [BP, LP, 4 * H], F32, kind="ExternalOutput")

    with tile.TileContext(nc) as tc, ExitStack() as ctx:
        cpool = ctx.enter_context(tc.tile_pool(name="consts", bufs=1))
        wp = ctx.enter_context(tc.tile_pool(name="work", bufs=2))
        pp = ctx.enter_context(tc.tile_pool(name="ps", bufs=2, space="PSUM"))
        ppg = ctx.enter_context(tc.tile_pool(name="psg", bufs=1, space="PSUM"))

        # ---- constants ----
        identf = cpool.tile([128, 128], F32)
        make_identity(nc, identf[:])
        identb = cpool.tile([128, 128], BF16)
        make_identity(nc, identb[:])
        onesMb = cpool.tile([1, 128], BF16)
        nc.vector.memset(onesMb[:], 1.0)
        ones_cb = cpool.tile([128, 1], BF16)
        nc.vector.memset(ones_cb[:], 1.0)
        Wt = cpool.tile([128, 2, H], F32)
        nc.sync.dma_start(Wt[:], w.rearrange("(k p) h -> p k h", p=128))

        # persistent glue PSUM bank: even/odd batches use column halves.
        # Per batch (base = (b%2)*256):
        #   nm1T   -> [0:1,  base     : base+100]   (dead before psHr starts)
        #   psNB   -> [0:100, base+100: base+101]   (dead before psHr starts)
        #   psSum  -> [0:1,  base+101 : base+102]   (rq extracted before psHr)
        #   psHr   -> [0:1,  base     : base+256]   (one iteration later)
        psTn = ppg.tile([128, 512], F32)

        # ---- prepass: q in both layouts, Aq = w^T q^T ----
        QtA = cpool.tile([128, 2, BP * LQ], F32)
        nc.sync.dma_start(QtA[:], qtg.rearrange("k p b l -> p k (b l)"))
        QnB = cpool.tile([128, BP, H], BF16)
        nc.sync.dma_start(QnB[0:LQ, :, :], qnb.rearrange("b l h -> l b h"))

        AqA = cpool.tile([128, 2, BP * LQ], F32)
        for gi in range(BP // 4):
            for ms in range(2):
                psAq = pp.tile([128, 400], F32, tag="pt")
                for kc in range(2):
                    nc.tensor.matmul(
                        psAq[:],
                        Wt[:, kc, ms * 128:(ms + 1) * 128],
                        QtA[:, kc, gi * 400:(gi + 1) * 400],
                        start=(kc == 0), stop=(kc == 1),
                    )
                dst = AqA[:, ms, gi * 400:(gi + 1) * 400]
                if ms == 0:
                    nc.scalar.copy(dst, psAq[:])
                else:
                    nc.vector.tensor_copy(dst, psAq[:])

        st = {b: {} for b in range(BP)}   # per-batch pipeline state

        def emit_load(b):
            Gt = wp.tile([128, NCH, 3 * H], F32, tag="Gt", bufs=7,
                         name=f"Gt{b}")
            nc.sync.dma_start(
                Gt[0:R, :, 0:H],
                pn[b].rearrange("(i r) h -> r i h", r=R))
            st[b]["Gt"] = Gt

        def stage_a1(b):
            # p^T via tensor transposes; copies to SBUF for the matmul lhsT
            Gt = st[b]["Gt"]
            psPt = [pp.tile([128, 400], F32, tag="pt", name=f"psPt{b}_{k}")
                    for k in range(2)]
            for kc in range(2):
                for i in range(NCH):
                    nc.tensor.transpose(
                        psPt[kc][0:128, i * R:(i + 1) * R],
                        Gt[0:R, i, kc * 128:(kc + 1) * 128],
                        identf[0:R, 0:R])
            PtS = wp.tile([128, 2, 400], F32, tag="PtS", name=f"PtS{b}")
            nc.scalar.copy(PtS[:, 0, :], psPt[0][:])
            nc.vector.tensor_copy(PtS[:, 1, :], psPt[1][:])
            st[b]["PtS"] = PtS

        def stage_a2(b):
            # S^T = p @ Aq  (LP chunks on partitions, LQ free), exact fp32
            PtS = st[b]["PtS"]
            bq = b * LQ
            psS = pp.tile([128, NCH, LQ], F32, tag="st", name=f"psS{b}")
            for i in range(NCH):
                for kc in range(2):
                    nc.tensor.matmul(
                        psS[0:R, i, :],
                        PtS[:, kc, i * R:(i + 1) * R],
                        AqA[:, kc, bq:bq + LQ],
                        start=(kc == 0), stop=(kc == 1),
                    )
            st[b]["psS"] = psS

        def stage_b1(b):
            # softmax stats, Q2C weights, e^T -> e transpose (all cheap/bf16)
            psS = st[b].pop("psS")
            Gt = st[b]["Gt"]
            base = (b % 2) * 256
            NM = wp.tile([128, NCH], F32, tag="NM", name=f"NM{b}")
            for i in range(NCH):
                nc.vector.tensor_reduce(
                    NM[0:R, i:i + 1], psS[0:R, i, :],
                    axis=AX.X, op=ALU.max, negate=True)
            E = wp.tile([128, NCH, LQ], BF16, tag="E", name=f"E{b}")
            RS = wp.tile([128, NCH], F32, tag="RS", name=f"RS{b}")
            for i in range(NCH):
                nc.scalar.activation(
                    E[0:R, i, :], psS[0:R, i, :], ACTF.Exp,
                    bias=NM[0:R, i:i + 1], accum_out=RS[0:R, i:i + 1])
            RCP = wp.tile([128, NCH], F32, tag="RCP", name=f"RCP{b}")
            nc.vector.reciprocal(RCP[0:R, :], RS[0:R, :])
            st[b]["RCP"] = RCP

            # global max g (pure stabilizer: bf16 rounding cancels)
            nm1 = wp.tile([128, 1], F32, tag="nm1", name=f"nm1{b}")
            nc.vector.tensor_reduce(nm1[0:R, :], NM[0:R, :], axis=AX.X,
                                    op=ALU.min)
            nc.tensor.transpose(psTn[0:1, base:base + R], nm1[0:R, 0:1],
                                identf[0:R, 0:R])
            ngb = wp.tile([1, 1], BF16, tag="ngb", name=f"ngb{b}")
            with nc.allow_low_precision(reason="softmax stabilizer cancels"):
                nc.vector.tensor_reduce(ngb[0:1, :],
                                        psTn[0:1, base:base + R],
                                        axis=AX.X, op=ALU.min)
            nc.tensor.matmul(psTn[0:R, base + 100:base + 101],
                             onesMb[0:1, 0:R], ngb[0:1, 0:1],
                             start=True, stop=True)
            nb = wp.tile([128, 1], F32, tag="nb", name=f"nb{b}")
            nc.scalar.copy(nb[0:R, :], psTn[0:R, base + 100:base + 101])
            EQ = wp.tile([128, NCH], BF16, tag="EQ", name=f"EQ{b}")
            nc.scalar.activation(EQ[0:R, :], NM[0:R, :], ACTF.Exp,
                                 bias=nb[0:R, 0:1], scale=-1.0)
            eqs = wp.tile([128, 1], BF16, tag="eqs", name=f"eqs{b}")
            with nc.allow_low_precision(reason="4-term sum, 2e-2 tolerance"):
                nc.vector.tensor_reduce(eqs[0:R, :], EQ[0:R, :], axis=AX.X,
                                        op=ALU.add)
            nc.tensor.matmul(psTn[0:1, base + 101:base + 102],
                             eqs[0:R, 0:1], ones_cb[0:R, 0:1],
                             start=True, stop=True)
            rq = wp.tile([1, 1], F32, tag="rq", name=f"rq{b}")
            nc.vector.reciprocal(rq[0:1, :],
                                 psTn[0:1, base + 101:base + 102])
            st[b]["EQ"] = EQ
            st[b]["rq"] = rq

            # bf16 copy of P for the h matmul
            Pb = wp.tile([128, NCH, H], BF16, tag="Pb", name=f"Pb{b}")
            nc.scalar.copy(Pb[0:R, 0, :], Gt[0:R, 0, 0:H])
            nc.scalar.copy(Pb[0:R, 1, :], Gt[0:R, 1, 0:H])
            nc.gpsimd.tensor_copy(Pb[0:R, 2, :], Gt[0:R, 2, 0:H])
            nc.gpsimd.tensor_copy(Pb[0:R, 3, :], Gt[0:R, 3, 0:H])
            st[b]["Pb"] = Pb

            # e^T -> e (bf16 transposes), SBUF copy for the U lhsT
            psAm = pp.tile([128, NCH, LQ], BF16, tag="am", bufs=1,
                           name=f"psAm{b}")
            for i in range(NCH):
                nc.tensor.transpose(
                    psAm[0:LQ, i, 0:R], E[0:R, i, :], identb[0:R, 0:R])
            Am = wp.tile([128, NCH, R], BF16, tag="Am", name=f"Am{b}")
            nc.vector.tensor_copy(Am[0:LQ, :, :], psAm[0:LQ, :, :])
            st[b]["Am"] = Am

        def stage_b2(b):
            # U chunks + fused normalize*P; h row (bf16) + its normalizer
            Gt = st[b]["Gt"]
            Am = st[b].pop("Am")
            RCP = st[b].pop("RCP")
            EQ = st[b].pop("EQ")
            Pb = st[b].pop("Pb")
            rq = st[b].pop("rq")
            base = (b % 2) * 256
            for i in range(NCH):
                psU = pp.tile([128, H], F32, tag="u", name=f"psU{b}_{i}")
                nc.tensor.matmul(
                    psU[0:R, :], Am[0:LQ, i, :], QnB[0:LQ, b, :],
                    start=True, stop=True)
                nc.vector.scalar_tensor_tensor(
                    Gt[0:R, i, H:2 * H], psU[0:R, :], RCP[0:R, i:i + 1],
                    Gt[0:R, i, 0:H], ALU.mult, ALU.mult)
            for i in range(NCH):
                nc.tensor.matmul(
                    psTn[0:1, base:base + H], EQ[0:R, i:i + 1],
                    Pb[0:R, i, :],
                    start=(i == 0), stop=(i == NCH - 1))
            hrowb = wp.tile([1, H], BF16, tag="hrowb", name=f"hrowb{b}")
            nc.scalar.mul(hrowb[0:1, :], psTn[0:1, base:base + H],
                          rq[0:1, 0:1])
            st[b]["hrowb"] = hrowb

        def stage_b3(b):
            # Ht broadcast, P*Ht, stores
            Gt = st[b].pop("Gt")
            hrowb = st[b].pop("hrowb")
            psHt = pp.tile([128, H], F32, tag="u", name=f"psHt{b}")
            nc.tensor.matmul(psHt[0:R, :], onesMb[0:1, 0:R],
                             hrowb[0:1, :], start=True, stop=True)
            HtS = wp.tile([128, 1, H], F32, tag="HtS", name=f"HtS{b}")
            nc.scalar.copy(HtS[0:R, 0, :], psHt[0:R, :])
            for i in range(NCH):
                eng = nc.gpsimd if i < 3 else nc.vector
                eng.tensor_tensor(
                    Gt[0:R, i, 2 * H:3 * H], Gt[0:R, i, 0:H],
                    HtS[0:R, 0, :], op=ALU.mult)
            gv = g[b].rearrange("(i r) h -> r i h", r=R)
            nc.sync.dma_start(gv[:, :, 0:H], Gt[0:R, :, 0:H])
            nc.sync.dma_start(
                gv[:, :, H:2 * H],
                HtS[0:R, 0:1, 0:H].broadcast_to([R, NCH, H]))
            nc.sync.dma_start(gv[:, :, 2 * H:4 * H], Gt[0:R, :, H:3 * H])

        # ---- pipelined driver ----
        emit_load(0)
        emit_load(1)
        for t in range(-3, BP + 1):
            if 0 <= t + 5 < BP:
                emit_load(t + 5)
            if 0 <= t + 3 < BP:
                stage_a1(t + 3)
            if 0 <= t + 2 < BP:
                stage_a2(t + 2)
            if 0 <= t < BP:
                stage_b2(t)
            if 0 <= t + 1 < BP:
                stage_b1(t + 1)
            if 0 <= t - 1 < BP:
                stage_b3(t - 1)

    return nc


def legalize_waits(nc):
    """Split multi-wait instructions into single-wait NoOps + instruction.

    The TPB ISA has exactly one (wait, update) EVENTS slot per 64B
    instruction; this walrus build refuses instructions with more than one
    sync wait ("Too many sync wait commands").  Tile's scheduler emits
    vector-clock waits freely, so legalize here: excess waits move onto
    engine-queue NoOps placed immediately before the instruction.
    """
    counter = 0
    for f in nc.m.functions:
        for blk in f.blocks:
            new = []
            for inst in blk.instructions:
                si = getattr(inst, "sync_info", None)
                if si is not None and len(si.on_wait) > 1:
                    waits = list(si.on_wait)
                    assert len(si.on_update) <= 1, inst
                    for wt in waits[:-1]:
                        counter += 1
                        new.append(mybir.InstNoOp(
                            name=f"I-waitnop-{counter}",
                            engine=inst.engine,
                            sync_info=mybir.SyncInfo(on_wait=[wt],
                                                     on_update=[]),
                        ))
                    inst.sync_info = mybir.SyncInfo(
                        on_wait=[waits[-1]], on_update=list(si.on_update))
                new.append(inst)
            blk.instructions = new
    return nc


def _make_in_maps(p, q, w):
    p = np.ascontiguousarray(p, dtype=np.float32)
    q = np.ascontiguousarray(q, dtype=np.float32)
    w = np.ascontiguousarray(w, dtype=np.float32)
    in_maps = []
    for c in range(NCORES):
        sl = slice(c * BP, (c + 1) * BP)
        qc = q[sl]
        in_maps.append({
            "pn": p[sl],
            "qtg": np.ascontiguousarray(
                qc.transpose(2, 0, 1).reshape(2, 128, BP, LQ)),
            "qnb": np.ascontiguousarray(qc.astype(ml_dtypes.bfloat16)),
            "w": w,
        })
    return in_maps


def run(p, q, w, trace=False):
    nc = legalize_waits(build_nc())
    res = run_bass_kernel_spmd(
        nc, _make_in_maps(p, q, w), list(range(NCORES)), trace=trace)
    out = np.concatenate([res.results[c]["g"] for c in range(NCORES)], axis=0)
    return out, res


def kernel(p, q, w):
    out, _ = run(p, q, w, trace=False)
    return out
